# revision 27
# baseline (speedup 1.0000x reference)
"""Trainium2 Bass kernel for nn_Net_50620484551136 (gnn_message_passing).

Network (see problem reference):
  h  = MLP(x)                     # 4652 -> 256 -> 256
  h1 = relu(GCN(h, e1)); h2 = relu(GCN(h, e2))
  h  = MLP([h1, h2])              # 512 -> 256 -> 256
  h1 = relu(GCN(h, e1)); h2 = relu(GCN(h, e2))
  h  = MLP([h1, h2])
  r1 = scatter_mean(h, index_1, N); r2 = scatter_mean(h, index_2, N)
  out = log_softmax(MLP([r1, r2]))

Strategy (8 NeuronCores, SPMD single program):
  - Tuple nodes sharded contiguously across cores (6250/core, padded to 6272).
  - All dense matmuls run feature-major (h^T: [feat, node]) in bf16, fp32 PSUM.
  - GCN: matmul commutes with aggregation; BOTH deg-norm factors
    (dinv[src]*dinv[dst]) are folded into the host-built one-hot SEG blocks,
    so the AllGathered payload is just h (node-major bf16, 256 wide).
    Each round: transpose h -> g_loc, AllGather to g_full [50176, 256],
    then each core gathers its incoming-edge rows (sorted by dst) with
    gpsimd.dma_gather and segment-sums them with PE matmuls against SEG.
  - dma_gather indices are int16, so gathers split into a low range
    (rows < 32768) and a high range; host pads each dst-tile's edge list to
    fixed per-tile lo/hi block counts so one static program serves all cores.
  - Conv outputs stay resident in SBUF (no DRAM roundtrip before the MLP).
  - Scatter-mean readout: each core segment-sums its OWN nodes into
    per-bin partials for ALL (padded) bins of both index relations
    (1/count folded into SEG), then one fp32 ReduceScatter hands every
    core its own bin shard.  Final MLP + log_softmax on device; host
    concatenates the 8 output shards.
"""

import numpy as np
import ml_dtypes

BF16 = ml_dtypes.bfloat16
FP8 = ml_dtypes.float8_e4m3
# payload dtype for the AllGathered h and the edge gathers (halves DMA)
GATHER_FP8 = True

# Problem constants (hardcoded per harness contract).
T = 50000
N_BINS = 5000
F_IN = 4652
DIM = 256
N_CLASSES = 5
NCORES = 8
SPLIT = 32768  # int16 gather index limit


def _ceil_to(x, m):
    return (x + m - 1) // m * m


def _wrap_idx(v):
    """int16 index vector (len % 16 == 0) -> [128, len/16] wrapped layout."""
    assert len(v) % 16 == 0
    w = v.reshape(-1, 16).T.astype(np.int16)  # [16, len/16]
    return np.tile(w, (8, 1))  # [128, len/16]


def _chunk_weight(w, dtype=BF16):
    """[K, M] -> [128, ceil(K/128), M] (partition = k%128, block = k//128)."""
    k, m = w.shape
    kp = _ceil_to(k, 128)
    wp = np.zeros((kp, m), np.float32)
    wp[:k] = w
    return np.ascontiguousarray(
        wp.reshape(kp // 128, 128, m).transpose(1, 0, 2)
    ).astype(dtype)


def _chunk_bias(b):
    """[M] -> [128, ceil(M/128)] f32 (partition = m%128, col = m//128)."""
    m = len(b)
    mp = _ceil_to(m, 128)
    bp = np.zeros(mp, np.float32)
    bp[:m] = b
    return np.ascontiguousarray(bp.reshape(mp // 128, 128).T).astype(np.float32)


def _prep_edges(src, dst, vals, dpc, dpad, spc, spad, ncores, seg_dtype):
    """Per-core gather indices + SEG blocks for one (src -> dst) relation.
    dst space is sharded dpc-per-core (padded dpad); src row id in the
    AllGathered buffer is (src//spc)*spad + src%spc.  Aggregation output for
    dst d is sum over edges e with dst==d of vals[e] * g[src_e].
    """
    nt = dpad // 128
    g_rows = ncores * spad
    has_hi = g_rows > SPLIT
    order = np.argsort(dst, kind="stable")
    src = src[order]
    dst = dst[order]
    vals = vals[order]
    core_of = dst // dpc
    gsrc = (src // spc) * spad + (src % spc)

    per_core = []
    nb_lo = 1
    nb_hi = 1 if has_hi else 0
    for p in range(ncores):
        sel = core_of == p
        sp = gsrc[sel]
        vv = vals[sel]
        ld = dst[sel] - p * dpc
        tiles = []
        for t in range(nt):
            m = (ld // 128) == t
            st = sp[m]
            vt = vv[m]
            dd = (ld[m] - t * 128).astype(np.int64)
            lo = st < SPLIT
            tiles.append((st[lo], st[~lo] - SPLIT, dd[lo], dd[~lo],
                          vt[lo], vt[~lo]))
            nb_lo = max(nb_lo, _ceil_to(max(len(dd[lo]), 1), 128) // 128)
            if has_hi:
                nb_hi = max(nb_hi, _ceil_to(max(len(dd[~lo]), 1), 128) // 128)
            else:
                assert len(dd[~lo]) == 0
        per_core.append(tiles)

    nb = nb_lo + nb_hi
    idx_arrs = []
    seg_arrs = []
    cnt_arrs = []
    for p in range(ncores):
        idx_a = np.zeros((nt, 128, nb * 8), np.int16)
        seg_a = np.zeros((nt, 128, nb * 128), np.float32)
        cnt_a = np.ones((1, nt * 2), np.int32)
        for t in range(nt):
            lo_gs, hi_gs, lo_dd, hi_dd, lo_v, hi_v = per_core[p][t]
            # pad with -1: descgen/DMA skip padded rows (count via reg)
            li = np.full(nb_lo * 128, -1, np.int64)
            li[: len(lo_gs)] = lo_gs
            if len(lo_gs) == 0:
                li[0] = 0
            cnt_a[0, 2 * t] = max(len(lo_gs), 1)
            idx_a[t, :, : nb_lo * 8] = _wrap_idx(li.astype(np.int16))
            if nb_hi:
                hi = np.full(nb_hi * 128, -1, np.int64)
                hi[: len(hi_gs)] = hi_gs
                if len(hi_gs) == 0:
                    hi[0] = 0
                cnt_a[0, 2 * t + 1] = max(len(hi_gs), 1)
                idx_a[t, :, nb_lo * 8 :] = _wrap_idx(hi.astype(np.int16))
            # seg[t, e%128, (e//128)*128 + dd] = edge value
            for off, dd_list, v_list in ((0, lo_dd, lo_v),
                                         (nb_lo * 128, hi_dd, hi_v)):
                i = np.arange(len(dd_list)) + off
                seg_a[t, i % 128, (i // 128) * 128 + dd_list] = v_list
        idx_arrs.append(idx_a)
        seg_arrs.append(np.ascontiguousarray(seg_a.astype(seg_dtype)))
        cnt_arrs.append(cnt_a)
    return dict(nb_lo=nb_lo, nb_hi=nb_hi, idx=idx_arrs, seg=seg_arrs,
                cnt=cnt_arrs)


def _prep_readout(idx1, invc1, idx2, invc2, tpc, bpc, brk, ncores, seg_dtype,
                  group=4):
    """Readout partial-sum prep: each core p scatter-adds its OWN nodes
    (local row ids, gathered from the local node-major h buffer) into the
    PADDED global bin layout
        rho = q*2*brk + (i-1)*brk + (b - q*bpc),   q = b // bpc
    for relation i bin b.  One unified edge list covers both relations.
    `group` consecutive tiles share one gather call (indices packed with
    0-pads, re-wrapped per group).
    """
    ntiles = ncores * 2 * brk // 128
    assert ntiles % group == 0
    per_core = []
    nb = 1
    for p in range(ncores):
        loc = np.arange(tpc, dtype=np.int64)
        rows = []
        for i, (idx, invc) in ((1, (idx1, invc1)), (2, (idx2, invc2))):
            b = idx[p * tpc : (p + 1) * tpc].astype(np.int64)
            q = b // bpc
            rho = q * 2 * brk + (i - 1) * brk + (b - q * bpc)
            rows.append(np.stack([loc, rho], axis=1))
        ed = np.concatenate(rows, axis=0)  # [2*tpc, 2] (local, rho)
        vals = np.concatenate([
            invc1[idx1[p * tpc : (p + 1) * tpc].astype(np.int64)],
            invc2[idx2[p * tpc : (p + 1) * tpc].astype(np.int64)],
        ])
        order = np.argsort(ed[:, 1], kind="stable")
        ed = ed[order]
        vals = vals[order]
        tiles = []
        for t in range(ntiles):
            m = (ed[:, 1] // 128) == t
            st = ed[m, 0]
            dd = ed[m, 1] - t * 128
            tiles.append((st, dd, vals[m]))
            nb = max(nb, _ceil_to(max(len(dd), 1), 128) // 128)
        per_core.append(tiles)

    ngroups = ntiles // group
    idx_arrs = []
    seg_arrs = []
    for p in range(ncores):
        idx_a = np.zeros((ngroups, 128, group * nb * 8), np.int16)
        seg_a = np.zeros((ntiles, 128, nb * 128), np.float32)
        for g in range(ngroups):
            li = np.zeros(group * nb * 128, np.int64)
            for k in range(group):
                t = g * group + k
                st, dd, vv = per_core[p][t]
                li[k * nb * 128 : k * nb * 128 + len(st)] = st
                i = np.arange(len(dd))
                seg_a[t, i % 128, (i // 128) * 128 + dd] = vv
            idx_a[g] = _wrap_idx(li.astype(np.int16))
        idx_arrs.append(idx_a)
        seg_arrs.append(np.ascontiguousarray(seg_a.astype(seg_dtype)))
    return dict(nb=nb, ntiles=ntiles, group=group, ngroups=ngroups,
                idx=idx_arrs, seg=seg_arrs)


def host_prep(inputs, ncores=NCORES, n_bins=None):
    """Pure-numpy preprocessing: sharding, edge sorting, SEG/idx construction,
    weight layout.  Only index arithmetic + data movement (no x-dependent
    compute)."""
    x = np.asarray(inputs["x"], np.float32)
    t_nodes, f_in = x.shape
    dim = np.asarray(inputs["W_i2"]).shape[0]
    ncls = np.asarray(inputs["b_fb"]).shape[0]
    if n_bins is None:
        if t_nodes == T and f_in == F_IN:
            n_bins = N_BINS
        else:
            n_bins = int(np.asarray(inputs["index_1"]).max()) + 1

    assert t_nodes % ncores == 0, (t_nodes, ncores)
    tpc = t_nodes // ncores
    tpad = _ceil_to(tpc, 128)
    nt = tpad // 128
    kin = _ceil_to(f_in, 128)
    assert n_bins % ncores == 0, (n_bins, ncores)
    bpc = n_bins // ncores
    bpad = _ceil_to(bpc, 128)
    bt = bpad // 128

    cfg = dict(
        t_nodes=t_nodes, f_in=f_in, dim=dim, ncls=ncls, n_bins=n_bins,
        ncores=ncores, tpc=tpc, tpad=tpad, nt=nt, kin=kin, kc=kin // 128,
        bpc=bpc, bpad=bpad, bt=bt, g_rows=ncores * tpad,
    )

    # ---- edge relations (with self-loops), both deg-norm factors in SEG
    rel = {}
    for r, key in ((1, "edge_index_1"), (2, "edge_index_2")):
        ei = np.asarray(inputs[key]).astype(np.int64)
        loop = np.arange(t_nodes, dtype=np.int64)
        s = np.concatenate([ei[0], loop])
        d = np.concatenate([ei[1], loop])
        deg = np.bincount(d, minlength=t_nodes).astype(np.float64)
        dinv = (1.0 / np.sqrt(np.maximum(deg, 1.0))).astype(np.float32)
        vals = dinv[s] * dinv[d]
        rel[r] = dict(
            prep=_prep_edges(s, d, vals, tpc, tpad, tpc, tpad, ncores,
                             FP8 if GATHER_FP8 else BF16),
        )
    cfg["rel"] = rel

    # ---- readout: local partial scatter-sum + ReduceScatter
    idx1 = np.asarray(inputs["index_1"]).astype(np.int64)
    idx2 = np.asarray(inputs["index_2"]).astype(np.int64)
    invc = []
    for idx in (idx1, idx2):
        cnt = np.bincount(idx, minlength=n_bins).astype(np.float64)
        invc.append((1.0 / np.maximum(cnt, 1.0)).astype(np.float32))
    cfg["ro"] = _prep_readout(idx1, invc[0], idx2, invc[1], tpc, bpc, bpad,
                              ncores, BF16)

    # ---- per-core x^T slices (bf16, padded)
    xT = []
    for p in range(ncores):
        xs = np.zeros((kin, tpad), np.float32)
        xs[:f_in, :tpc] = x[p * tpc : (p + 1) * tpc].T
        xT.append(np.ascontiguousarray(xs).astype(BF16))
    cfg["xT"] = xT

    # ---- weights
    w = {}
    w["wi1"] = _chunk_weight(np.asarray(inputs["W_i1"], np.float32))
    w["wi2"] = _chunk_weight(np.asarray(inputs["W_i2"], np.float32))
    for nm, src in (("wc11", "Wc11"), ("wc12", "Wc12"),
                    ("wc21", "Wc21"), ("wc22", "Wc22"),
                    ("wm1a", "W_m1a"), ("wm1b", "W_m1b"),
                    ("wm2a", "W_m2a"), ("wm2b", "W_m2b"),
                    ("wfa", "W_fa"), ("wfb", "W_fb")):
        w[nm] = _chunk_weight(np.asarray(inputs[src], np.float32))
    for nm, src in (("bi1", "b_i1"), ("bi2", "b_i2"),
                    ("bc11", "bc11"), ("bc12", "bc12"),
                    ("bc21", "bc21"), ("bc22", "bc22"),
                    ("bm1a", "b_m1a"), ("bm1b", "b_m1b"),
                    ("bm2a", "b_m2a"), ("bm2b", "b_m2b"),
                    ("bfa", "b_fa"), ("bfb", "b_fb")):
        w[nm] = _chunk_bias(np.asarray(inputs[src], np.float32))
    w["ident16"] = np.eye(128, dtype=BF16)
    w["ident32"] = np.eye(128, dtype=np.float32)
    cfg["w"] = w
    return cfg


def _nchunks(total, step):
    out = []
    o = 0
    while o < total:
        out.append((o, min(step, total - o)))
        o += step
    return out


def build_program(cfg):
    """Build the SPMD bass program (one program, 8 cores)."""
    import concourse.bass as bass
    import concourse.mybir as mybir
    import concourse.tile as tile
    from concourse import bacc

    dt = mybir.dt
    AF = mybir.ActivationFunctionType
    ALU = mybir.AluOpType

    nt, tpad, kc = cfg["nt"], cfg["tpad"], cfg["kc"]
    bt, bpad = cfg["bt"], cfg["bpad"]
    dim, ncls = cfg["dim"], cfg["ncls"]
    dc = dim // 128
    g_rows = cfg["g_rows"]
    ncores = cfg["ncores"]
    rel, ro = cfg["rel"], cfg["ro"]
    rg = [list(range(ncores))]
    ro_tiles, ro_nb = ro["ntiles"], ro["nb"]
    ro_grp, ro_ngrp = ro["group"], ro["ngroups"]
    pay_dt = dt.float8e4 if GATHER_FP8 else dt.bfloat16

    nc = bacc.Bacc("TRN2", target_bir_lowering=False, debug=False,
                   num_devices=ncores, num_swdge_queues=4)
    qstate = [0]

    def next_q():
        q = qstate[0]
        qstate[0] = (q + 1) % 4
        return q

    # rotating register pool for runtime gather counts (bounds the number
    # of concurrently-live registers that the scheduler can hoist)
    cnt_regs = [nc.gpsimd.alloc_register(f"cntreg{i}") for i in range(8)]
    rstate = [0]

    def load_cnt(ap):
        r = cnt_regs[rstate[0] % len(cnt_regs)]
        rstate[0] += 1
        nc.gpsimd.reg_load(r, ap)
        return r

    # ---------------- I/O declarations ----------------
    xT = nc.dram_tensor("xT", [cfg["kin"], tpad], dt.bfloat16,
                        kind="ExternalInput")
    seg_in, idx_in, cnt_in = {}, {}, {}
    for r in (1, 2):
        pr = rel[r]["prep"]
        nb = pr["nb_lo"] + pr["nb_hi"]
        seg_in[r] = nc.dram_tensor(f"seg{r}", [nt, 128, nb * 128], pay_dt,
                                   kind="ExternalInput")
        idx_in[r] = nc.dram_tensor(f"idx{r}", [nt, 128, nb * 8], dt.int16,
                                   kind="ExternalInput")
        cnt_in[r] = nc.dram_tensor(f"cnt{r}", [1, nt * 2], dt.int32,
                                   kind="ExternalInput")
    segro_in = nc.dram_tensor("segro", [ro_tiles, 128, ro_nb * 128],
                              dt.bfloat16, kind="ExternalInput")
    idxro_in = nc.dram_tensor("idxro", [ro_ngrp, 128, ro_grp * ro_nb * 8],
                              dt.int16, kind="ExternalInput")

    wnames_bf = dict(
        wi1=[128, kc, dim], wi2=[128, dc, dim],
        wc11=[128, dc, dim], wc12=[128, dc, dim],
        wc21=[128, dc, dim], wc22=[128, dc, dim],
        wm1a=[128, 2 * dc, dim], wm1b=[128, dc, dim],
        wm2a=[128, 2 * dc, dim], wm2b=[128, dc, dim],
        wfa=[128, 2 * dc, dim], wfb=[128, dc, ncls],
        ident16=[128, 128],
    )
    wnames_f32 = dict(
        bi1=[128, dc], bi2=[128, dc],
        bc11=[128, dc], bc12=[128, dc], bc21=[128, dc], bc22=[128, dc],
        bm1a=[128, dc], bm1b=[128, dc], bm2a=[128, dc], bm2b=[128, dc],
        bfa=[128, dc], bfb=[128, 1],
        ident32=[128, 128],
    )
    win = {}
    for nm, shp in wnames_bf.items():
        win[nm] = nc.dram_tensor(nm, shp, dt.bfloat16, kind="ExternalInput")
    for nm, shp in wnames_f32.items():
        win[nm] = nc.dram_tensor(nm, shp, dt.float32, kind="ExternalInput")

    out_dram = nc.dram_tensor("out", [bpad, ncls], dt.float32,
                              kind="ExternalOutput")

    nb_max = max(
        max(rel[r]["prep"]["nb_lo"] + rel[r]["prep"]["nb_hi"] for r in (1, 2)),
        ro_nb,
    )

    with tile.TileContext(nc) as tc:
        with (
            tc.tile_pool(name="wpool", bufs=1) as wpool,
            tc.tile_pool(name="hpool", bufs=1) as hpool,
            tc.tile_pool(name="hcpool", bufs=1) as hcpool,
            tc.tile_pool(name="xpool", bufs=4) as xpool,
            tc.tile_pool(name="edpool", bufs=3) as edpool,
            tc.tile_pool(name="segpool", bufs=2) as segpool,
            tc.tile_pool(name="idxpool", bufs=2) as idxpool,
            tc.tile_pool(name="apool", bufs=4) as apool,
            tc.tile_pool(name="gpool", bufs=3) as gpool,
            tc.tile_pool(name="mpool", bufs=4) as mpool,
            tc.tile_pool(name="pbig", bufs=3, space="PSUM") as pbig,
            tc.tile_pool(name="pagg", bufs=2, space="PSUM") as pagg,
            tc.tile_pool(name="pcnv", bufs=3, space="PSUM") as pcnv,
            tc.tile_pool(name="dpool", bufs=1, space="DRAM") as dpool,
        ):
            # ---- resident weights
            wsb = {}
            for nm in list(wnames_bf) + list(wnames_f32):
                shp = wnames_bf.get(nm) or wnames_f32[nm]
                dtyp = dt.bfloat16 if nm in wnames_bf else dt.float32
                wt = wpool.tile(shp, dtyp, name=f"sb_{nm}", tag=f"w_{nm}")
                nc.sync.dma_start(wt[:], win[nm][:])
                wsb[nm] = wt
            cnt_sb = {}
            for r in (1, 2):
                ct = wpool.tile([1, nt * 2], dt.int32, name=f"sb_cnt{r}",
                                tag=f"w_cnt{r}")
                nc.sync.dma_start(ct[:], cnt_in[r][:])
                cnt_sb[r] = ct
            # prime the gather-output slots: skipped (padded) rows leave
            # stale SBUF, which must be finite so SEG zeros neutralize it
            for _ in range(3):
                edt = edpool.tile([128, nb_max, dim], pay_dt,
                                  name="ed", tag="ed")
                nc.vector.memset(edt[:], 0.0)
            for _ in range(2):
                edt = edpool.tile([128, ro_grp * ro_nb, dim], dt.bfloat16,
                                  name="edr", tag="edr")
                nc.vector.memset(edt[:], 0.0)

            # =========== Phase 1: input MLP  h0 = relu(x@Wi1+bi1)@Wi2+bi2
            h_cur = hpool.tile([128, dc, tpad], dt.bfloat16, name="h0T",
                               tag="hT")
            for (n0, nw) in _nchunks(tpad, 512):
                ps1 = []
                for f in range(dc):
                    p_ = pbig.tile([128, 512], dt.float32, name=f"ps1_{f}",
                                   tag="mlp")
                    ps1.append(p_)
                for k in range(kc):
                    xt = xpool.tile([128, 512], dt.bfloat16, name="xt",
                                    tag="xt")
                    nc.sync.dma_start(xt[:, :nw],
                                      xT[k * 128:(k + 1) * 128, n0:n0 + nw])
                    for f in range(dc):
                        nc.tensor.matmul(
                            ps1[f][:, :nw],
                            lhsT=wsb["wi1"][:, k, f * 128:(f + 1) * 128],
                            rhs=xt[:, :nw],
                            start=(k == 0), stop=(k == kc - 1))
                a1 = []
                for f in range(dc):
                    a_ = apool.tile([128, 512], dt.bfloat16, name=f"a1_{f}",
                                    tag="a1")
                    nc.scalar.activation(a_[:, :nw], ps1[f][:, :nw], AF.Relu,
                                         bias=wsb["bi1"][:, f:f + 1])
                    a1.append(a_)
                for f2 in range(dc):
                    p2 = pbig.tile([128, 512], dt.float32, name="ps2",
                                   tag="mlp")
                    for k2 in range(dc):
                        nc.tensor.matmul(
                            p2[:, :nw],
                            lhsT=wsb["wi2"][:, k2, f2 * 128:(f2 + 1) * 128],
                            rhs=a1[k2][:, :nw],
                            start=(k2 == 0), stop=(k2 == dc - 1))
                    nc.vector.tensor_scalar(
                        h_cur[:, f2, n0:n0 + nw], p2[:, :nw],
                        wsb["bi2"][:, f2:f2 + 1], None, ALU.add)

            # =========== Phase 2: two GCN rounds
            for rnd in (1, 2):
                # ---- a) g_loc = node-major h [tpad, dim]
                g_loc = dpool.tile([tpad, dim], pay_dt, name="g_loc",
                                   tag=f"g_loc{rnd}")
                for t in range(nt):
                    gt = gpool.tile([128, dim], pay_dt, name="gt",
                                    tag="gt")
                    for f in range(dc):
                        tp = pcnv.tile([128, 128], dt.bfloat16, name="trp",
                                       tag="cnv")
                        nc.tensor.transpose(
                            tp[:], h_cur[:, f, t * 128:(t + 1) * 128],
                            wsb["ident16"][:])
                        nc.vector.tensor_copy(gt[:, f * 128:(f + 1) * 128],
                                              tp[:])
                    nc.sync.dma_start(g_loc[t * 128:(t + 1) * 128, :], gt[:])
                g_full = dpool.tile([g_rows, dim], pay_dt,
                                    name="g_full", tag=f"g_full{rnd}")
                nc.gpsimd.collective_compute(
                    "AllGather", ALU.bypass, replica_groups=rg,
                    ins=[g_loc[:]], outs=[g_full[:]])

                # ---- b) two conv relations; outputs stay in SBUF
                houts = []
                for r in (1, 2):
                    pr = rel[r]["prep"]
                    nb_lo, nb_hi = pr["nb_lo"], pr["nb_hi"]
                    nb = nb_lo + nb_hi
                    wc = wsb[f"wc{rnd}{r}"]
                    bc = wsb[f"bc{rnd}{r}"]
                    hout = hcpool.tile([128, dc, tpad], dt.bfloat16,
                                       name=f"h{r}T", tag=f"hc_{r}")
                    for t in range(nt):
                        idxt = idxpool.tile([128, nb_max * 8], dt.int16,
                                            name="idxt", tag="idx")
                        nc.sync.dma_start(idxt[:, :nb * 8], idx_in[r][t])
                        segt = segpool.tile([128, nb_max * 128], pay_dt,
                                            name="segt", tag="seg")
                        nc.sync.dma_start(segt[:, :nb * 128], seg_in[r][t])
                        ed = edpool.tile([128, nb_max, dim], pay_dt,
                                         name="ed", tag="ed")
                        nlo = load_cnt(cnt_sb[r][0:1, 2 * t:2 * t + 1])
                        nc.gpsimd.dma_gather(
                            ed[:, 0:nb_lo, :], g_full[:],
                            idxt[:, 0:nb_lo * 8],
                            nb_lo * 128, nlo, dim,
                            single_packet=False, queue_num=next_q())
                        if nb_hi:
                            nhi = load_cnt(
                                cnt_sb[r][0:1, 2 * t + 1:2 * t + 2])
                            nc.gpsimd.dma_gather(
                                ed[:, nb_lo:nb, :], g_full[SPLIT:g_rows, :],
                                idxt[:, nb_lo * 8:nb * 8],
                                nb_hi * 128, nhi, dim,
                                single_packet=False, queue_num=next_q())
                        # segment-sum: SEG stationary, gathered rows moving
                        agg = pagg.tile([128, dim], dt.float32, name="agg",
                                        tag="agg")
                        for b in range(nb):
                            nc.tensor.matmul(
                                agg[:],
                                lhsT=segt[:, b * 128:(b + 1) * 128],
                                rhs=ed[:, b, :],
                                start=(b == 0), stop=(b == nb - 1))
                        aggs = mpool.tile([128, dim], dt.bfloat16, name="aggs",
                                          tag="aggs")
                        nc.vector.tensor_copy(aggs[:], agg[:])
                        # transpose to feature-major for the conv matmul
                        aggT = mpool.tile([128, dim], dt.bfloat16,
                                          name="aggT", tag="aggT")
                        for f in range(dc):
                            tp = pcnv.tile([128, 128], dt.bfloat16,
                                           name="tpc", tag="cnv")
                            nc.tensor.transpose(
                                tp[:], aggs[:, f * 128:(f + 1) * 128],
                                wsb["ident16"][:])
                            nc.vector.tensor_copy(
                                aggT[:, f * 128:(f + 1) * 128], tp[:])
                        for f2 in range(dc):
                            cps = pcnv.tile([128, 128], dt.float32,
                                            name=f"cps{f2}", tag="cnv")
                            for k in range(dc):
                                nc.tensor.matmul(
                                    cps[:],
                                    lhsT=wc[:, k, f2 * 128:(f2 + 1) * 128],
                                    rhs=aggT[:, k * 128:(k + 1) * 128],
                                    start=(k == 0), stop=(k == dc - 1))
                            nc.vector.tensor_scalar(
                                hout[:, f2, t * 128:(t + 1) * 128],
                                cps[:],
                                bc[:, f2:f2 + 1], 0.0, ALU.add, ALU.max)
                    houts.append(hout)

                # ---- c) mlp_rnd on concat(h1, h2) straight from SBUF
                wma = wsb[f"wm{rnd}a"]
                wmb = wsb[f"wm{rnd}b"]
                bma = wsb[f"bm{rnd}a"]
                bmb = wsb[f"bm{rnd}b"]
                h_next = hpool.tile([128, dc, tpad], dt.bfloat16,
                                    name=f"hm{rnd}T", tag="hT")
                for (n0, nw) in _nchunks(tpad, 512):
                    ps1 = []
                    for f in range(dc):
                        p_ = pbig.tile([128, 512], dt.float32, name="psm1",
                                       tag="mlp")
                        ps1.append(p_)
                    for k in range(2 * dc):
                        rhs_src = houts[0] if k < dc else houts[1]
                        for f in range(dc):
                            nc.tensor.matmul(
                                ps1[f][:, :nw],
                                lhsT=wma[:, k, f * 128:(f + 1) * 128],
                                rhs=rhs_src[:, k % dc, n0:n0 + nw],
                                start=(k == 0), stop=(k == 2 * dc - 1))
                    am = []
                    for f in range(dc):
                        a_ = apool.tile([128, 512], dt.bfloat16, name="am",
                                        tag="a1")
                        nc.scalar.activation(a_[:, :nw], ps1[f][:, :nw],
                                             AF.Relu, bias=bma[:, f:f + 1])
                        am.append(a_)
                    for f2 in range(dc):
                        p2 = pbig.tile([128, 512], dt.float32, name="psm2",
                                       tag="mlp")
                        for k2 in range(dc):
                            nc.tensor.matmul(
                                p2[:, :nw],
                                lhsT=wmb[:, k2, f2 * 128:(f2 + 1) * 128],
                                rhs=am[k2][:, :nw],
                                start=(k2 == 0), stop=(k2 == dc - 1))
                        nc.vector.tensor_scalar(
                            h_next[:, f2, n0:n0 + nw], p2[:, :nw],
                            bmb[:, f2:f2 + 1], None, ALU.add)
                h_cur = h_next

            # =========== Phase 3: readout
            # a) node-major final h (local only)
            hf_loc = dpool.tile([tpad, dim], dt.bfloat16, name="hf_loc",
                                tag="hf_loc")
            for t in range(nt):
                gt = gpool.tile([128, dim], dt.bfloat16, name="gtf",
                                tag="gt")
                for f in range(dc):
                    tp = pcnv.tile([128, 128], dt.bfloat16, name="trpf",
                                   tag="cnv")
                    nc.tensor.transpose(
                        tp[:], h_cur[:, f, t * 128:(t + 1) * 128],
                        wsb["ident16"][:])
                    nc.vector.tensor_copy(gt[:, f * 128:(f + 1) * 128], tp[:])
                nc.sync.dma_start(hf_loc[t * 128:(t + 1) * 128, :], gt[:])

            # b) local partial scatter-sums over the padded bin layout
            # (group ro_grp tiles per gather call; 0-index padding)
            rs_in = dpool.tile([ro_tiles * 128, dim], dt.float32,
                               name="rs_in", tag="rs_in")
            for g in range(ro_ngrp):
                idxt = idxpool.tile([128, ro_grp * ro_nb * 8], dt.int16,
                                    name="idxtr", tag="idxr")
                nc.sync.dma_start(idxt[:], idxro_in[g])
                ed = edpool.tile([128, ro_grp * ro_nb, dim], dt.bfloat16,
                                 name="edr", tag="edr")
                nc.gpsimd.dma_gather(
                    ed[:], hf_loc[:],
                    idxt[:],
                    ro_grp * ro_nb * 128, ro_grp * ro_nb * 128, dim,
                    single_packet=False, queue_num=next_q())
                for k in range(ro_grp):
                    t = g * ro_grp + k
                    segt = segpool.tile([128, nb_max * 128], dt.bfloat16,
                                        name="segtr", tag="segr")
                    nc.sync.dma_start(segt[:, :ro_nb * 128], segro_in[t])
                    agg = pagg.tile([128, dim], dt.float32, name="aggr",
                                    tag="agg")
                    for b in range(ro_nb):
                        nc.tensor.matmul(
                            agg[:],
                            lhsT=segt[:, b * 128:(b + 1) * 128],
                            rhs=ed[:, k * ro_nb + b, :],
                            start=(b == 0), stop=(b == ro_nb - 1))
                    aggs = mpool.tile([128, dim], dt.float32, name="aggsr",
                                      tag="aggs32")
                    nc.vector.tensor_copy(aggs[:], agg[:])
                    nc.sync.dma_start(rs_in[t * 128:(t + 1) * 128, :],
                                      aggs[:])

            rs_out = dpool.tile([2 * bpad, dim], dt.float32, name="rs_out",
                                tag="rs_out")
            nc.gpsimd.collective_compute(
                "ReduceScatter", ALU.add, replica_groups=rg,
                ins=[rs_in[:]], outs=[rs_out[:]])

            # c) transpose own shard to feature-major rcat [128, 2*dc, bpad]
            rcat = hpool.tile([128, 2 * dc, bpad], dt.bfloat16, name="rcat",
                              tag="rcat")
            for i in (1, 2):
                for t in range(bt):
                    rld = mpool.tile([128, dim], dt.float32, name="rld",
                                     tag="rld")
                    nc.sync.dma_start(
                        rld[:],
                        rs_out[(i - 1) * bpad + t * 128:
                               (i - 1) * bpad + (t + 1) * 128, :])
                    for f in range(dc):
                        tp = pcnv.tile([128, 128], dt.float32,
                                       name="tpr", tag="cnv")
                        nc.tensor.transpose(
                            tp[:], rld[:, f * 128:(f + 1) * 128],
                            wsb["ident32"][:])
                        nc.vector.tensor_copy(
                            rcat[:, (i - 1) * dc + f, t * 128:(t + 1) * 128],
                            tp[:])

            # d) final MLP + log_softmax
            logitsT = hpool.tile([128, bpad], dt.float32, name="logitsT",
                                 tag="logitsT")
            nc.vector.memset(logitsT[:], 0.0)
            for (n0, nw) in _nchunks(bpad, 512):
                ps1 = []
                for f in range(dc):
                    p_ = pbig.tile([128, 512], dt.float32, name="psf1",
                                   tag="mlp")
                    ps1.append(p_)
                for k in range(2 * dc):
                    for f in range(dc):
                        nc.tensor.matmul(
                            ps1[f][:, :nw],
                            lhsT=wsb["wfa"][:, k, f * 128:(f + 1) * 128],
                            rhs=rcat[:, k, n0:n0 + nw],
                            start=(k == 0), stop=(k == 2 * dc - 1))
                af = []
                for f in range(dc):
                    a_ = apool.tile([128, 512], dt.bfloat16, name="af",
                                    tag="a1")
                    nc.scalar.activation(a_[:, :nw], ps1[f][:, :nw], AF.Relu,
                                         bias=wsb["bfa"][:, f:f + 1])
                    af.append(a_)
                pl = pbig.tile([128, 512], dt.float32, name="psl", tag="mlp")
                for k2 in range(dc):
                    nc.tensor.matmul(
                        pl[:ncls, :nw],
                        lhsT=wsb["wfb"][:, k2, :ncls],
                        rhs=af[k2][:, :nw],
                        start=(k2 == 0), stop=(k2 == dc - 1))
                nc.vector.tensor_scalar(
                    logitsT[:ncls, n0:n0 + nw], pl[:ncls, :nw],
                    wsb["bfb"][:ncls, 0:1], None, ALU.add)

            for t in range(bt):
                ltp = pcnv.tile([128, 128], dt.float32, name="ltp", tag="cnv")
                nc.tensor.transpose(
                    ltp[:], logitsT[:, t * 128:(t + 1) * 128],
                    wsb["ident32"][:])
                mx = mpool.tile([128, 1], dt.float32, name="mx", tag="mx")
                nc.vector.tensor_reduce(mx[:], ltp[:, :ncls],
                                        mybir.AxisListType.X, ALU.max)
                z = mpool.tile([128, ncls], dt.float32, name="z", tag="z")
                nc.vector.tensor_scalar(z[:], ltp[:, :ncls], mx[:, 0:1], None,
                                        ALU.subtract)
                ez = mpool.tile([128, ncls], dt.float32, name="ez", tag="z")
                nc.scalar.activation(ez[:], z[:], AF.Exp)
                sm = mpool.tile([128, 1], dt.float32, name="sm", tag="mx")
                nc.vector.tensor_reduce(sm[:], ez[:], mybir.AxisListType.X,
                                        ALU.add)
                ls = mpool.tile([128, 1], dt.float32, name="ls", tag="mx")
                nc.scalar.activation(ls[:], sm[:], AF.Ln)
                o = mpool.tile([128, ncls], dt.float32, name="o", tag="z")
                nc.vector.tensor_scalar(o[:], z[:], ls[:, 0:1], None,
                                        ALU.subtract)
                nc.sync.dma_start(out_dram[t * 128:(t + 1) * 128, :], o[:])

    nc.compile()
    return nc


def build_in_maps(cfg):
    in_maps = []
    for p in range(cfg["ncores"]):
        m = dict(
            xT=cfg["xT"][p],
            seg1=cfg["rel"][1]["prep"]["seg"][p],
            idx1=cfg["rel"][1]["prep"]["idx"][p],
            cnt1=cfg["rel"][1]["prep"]["cnt"][p],
            seg2=cfg["rel"][2]["prep"]["seg"][p],
            idx2=cfg["rel"][2]["prep"]["idx"][p],
            cnt2=cfg["rel"][2]["prep"]["cnt"][p],
            segro=cfg["ro"]["seg"][p],
            idxro=cfg["ro"]["idx"][p],
        )
        m.update({k: v for k, v in cfg["w"].items()})
        in_maps.append(m)
    return in_maps


_CACHE = {}


def kernel(**inputs) -> np.ndarray:
    cfg = host_prep(inputs)
    key = (
        cfg["t_nodes"], cfg["f_in"], cfg["dim"], cfg["ncls"], cfg["n_bins"],
        tuple((cfg["rel"][r]["prep"]["nb_lo"], cfg["rel"][r]["prep"]["nb_hi"])
              for r in (1, 2)),
        (cfg["ro"]["nb"], cfg["ro"]["ntiles"]),
    )
    if key not in _CACHE:
        _CACHE[key] = build_program(cfg)
    nc = _CACHE[key]

    from concourse.bass_utils import run_bass_kernel_spmd

    in_maps = build_in_maps(cfg)
    res = run_bass_kernel_spmd(nc, in_maps, list(range(cfg["ncores"])))
    outs = [res.results[p]["out"][: cfg["bpc"]] for p in range(cfg["ncores"])]
    return np.ascontiguousarray(np.concatenate(outs, axis=0), np.float32)
